# revision 1
# baseline (speedup 1.0000x reference)
"""Quantized int8 matmul on 8 TRN2 NeuronCores.

Math: out = ((x - ZP_X) * SCALE_X) @ ((y - ZP_Y) * SCALE_Y)
Implemented as: out = [(x - ZP_X) @ (y - ZP_Y)] * (SCALE_X * SCALE_Y)
The zero-point-shifted int8 values (range ~[-150, 155]) are exactly
representable in bf16, so a bf16 matmul with fp32 PSUM accumulation is
numerically ~identical to the fp32 reference.

Sharding: x row-sharded (M) across 8 cores, y replicated, no collectives.
Each core's x shard is laid out [K, m_loc] in DRAM (layout chosen at
shard time on host) so the TensorE stationary operand [k-part, m-free]
loads directly -- no on-device transpose.

Engine split per core:
  PE     - warm-up dummies + 1024 matmuls (128k x 128m x 512n)
  SP     - HWDGE y loads, first x loads, out store
  GpSimd - SWDGE x loads (keeps the SP sequencer free), warmup memsets
  DVE    - x int8->bf16 +25 converts, y converts (even batches, nb>0),
           odd-mt PSUM evictions
  ACT    - y converts (all of nb=0; odd batches after) + even-mt evictions
"""

import numpy as np

SCALE_X, ZP_X = 0.0215, -25
SCALE_Y, ZP_Y = 0.0176, 18
M, K, N = 4096, 4096, 4096
N_CORES = 8
P = 128
NBLK = 512  # matmul free dim = one PSUM bank of fp32
KB = 2  # k-tiles per y DMA/convert batch
XB = 2  # k-tiles per x DMA/convert batch
N_WARM = 13  # PE warm-up dummy matmuls


def build_nc(m_loc, k, n):
    from contextlib import ExitStack

    import concourse.mybir as mybir
    import concourse.tile as tile
    from concourse import bacc
    from concourse.bass import ds, ts

    fp32 = mybir.dt.float32
    bf16 = mybir.dt.bfloat16
    int8 = mybir.dt.int8
    Copy = mybir.ActivationFunctionType.Copy

    MT = m_loc // P  # m tiles (PE stationary free dim blocks)
    KT = k // P  # contraction tiles
    NB = n // NBLK  # output column blocks
    kb = min(KB, KT)  # y batch size in k-tiles
    xb = min(XB, KT)  # x batch size in k-tiles
    NKB = KT // kb
    NXB = KT // xb

    nc = bacc.Bacc(None, debug=False)
    xt = nc.declare_dram_parameter("xt", [k, m_loc], int8, isOutput=False)
    y = nc.declare_dram_parameter("y", [k, n], int8, isOutput=False)
    out = nc.declare_dram_parameter("out", [m_loc, n], fp32, isOutput=True)

    # Batched DRAM views: group k into (batch, tile-in-batch, partition)
    xt_r = xt.rearrange("(g b p) m -> g p b m", b=xb, p=P)
    xt_r1 = xt.rearrange("(t p) m -> t p m", p=P)
    y_r = y.rearrange("(q b p) n -> q p b n", b=kb, p=P)

    with ExitStack() as ctx:
        tc = ctx.enter_context(tile.TileContext(nc))
        wm_pool = ctx.enter_context(tc.tile_pool(name="wm", bufs=1))
        xi_pool = ctx.enter_context(tc.tile_pool(name="xi", bufs=4))
        xt_pool = ctx.enter_context(tc.tile_pool(name="xtb", bufs=1, side="right"))
        yi_pool = ctx.enter_context(tc.tile_pool(name="yi", bufs=12))
        yb_pool = ctx.enter_context(tc.tile_pool(name="yb", bufs=12, side="right"))
        ob_pool = ctx.enter_context(tc.tile_pool(name="ob", bufs=4))
        ps_pool = ctx.enter_context(tc.tile_pool(name="ps", bufs=8, space="PSUM"))

        # PE warm-up: a few dummy matmuls on zeroed tiles, issued during
        # the startup DMA window so the HAM clock-gate opens before the
        # real matmul stream begins.
        wm_w = wm_pool.tile([P, P], bf16)
        wm_s = wm_pool.tile([P, NBLK], bf16)
        nc.gpsimd.memset(wm_w[:], 0.0)
        nc.gpsimd.memset(wm_s[:], 0.0)
        ps_warm = ps_pool.tile([P, NBLK], fp32, tag="ps", name="warm")
        for _ in range(N_WARM):
            nc.tensor.matmul(ps_warm[:], wm_w[:], wm_s[:], start=True, stop=True)

        # Persistent bf16 x^T: partition = k within tile, free = (kt, m)
        xT = xt_pool.tile([P, KT, m_loc], bf16)

        def emit_x(g):
            if g >= NXB:
                return
            if g == 0 and xb > 1:
                # first batch as single k-tiles on the fast HWDGE queue so
                # the very first matmul's weights land ASAP
                for t in range(xb):
                    xi = xi_pool.tile([P, m_loc], int8, name=f"xi0_{t}", tag="xi1")
                    nc.sync.dma_start(xi[:], xt_r1[t])
                    nc.vector.tensor_scalar_add(xT[:, t, :], xi[:], float(-ZP_X))
                return
            xi = xi_pool.tile([P, xb, m_loc], int8, name=f"xi_{g}")
            eng = nc.sync if g < 2 else nc.gpsimd
            eng.dma_start(xi[:], xt_r[g])
            nc.vector.tensor_scalar_add(xT[:, ts(g, xb), :], xi[:], float(-ZP_X))

        # Final N-block split into two half-width blocks: the very last
        # eviction + out-DMA drain (which the exit barrier waits on) is
        # half as long.
        blocks = [(i * NBLK, NBLK) for i in range(NB - 1)]
        if NBLK % 2 == 0:
            half = NBLK // 2
            blocks += [((NB - 1) * NBLK, half), ((NB - 1) * NBLK + half, half)]
        else:
            blocks += [((NB - 1) * NBLK, NBLK)]

        for bi, (col, w) in enumerate(blocks):
            psums = [
                ps_pool.tile([P, w], fp32, tag="ps", name=f"acc_{bi}_{i}")
                for i in range(MT)
            ]
            if bi == 0:
                # x batches 0/1 go out first (they gate the first matmuls)
                emit_x(0)
                emit_x(1)
            for q in range(NKB):
                if bi == 0 and q > 0:
                    emit_x(q + 1)
                yi = yi_pool.tile([P, kb, w], int8, name=f"yi_{bi}_{q}", tag="yi")
                nc.sync.dma_start(yi[:], y_r[q, :, :, ds(col, w)])
                yb = yb_pool.tile([P, kb, w], bf16, name=f"yb_{bi}_{q}", tag="yb")
                if bi > 0 and q % 2 == 0:
                    nc.vector.tensor_scalar_add(yb[:], yi[:], float(-ZP_Y))
                else:
                    nc.scalar.activation(yb[:], yi[:], Copy, bias=float(-ZP_Y))
                for kti in range(kb):
                    kt = q * kb + kti
                    for mt in range(MT):
                        nc.tensor.matmul(
                            psums[mt][:],
                            xT[:, kt, ts(mt, P)],
                            yb[:, kti, :],
                            start=(kt == 0),
                            stop=(kt == KT - 1),
                        )
            for mt in range(MT):
                ob = ob_pool.tile([P, w], fp32, name=f"ob_{bi}_{mt}", tag="ob")
                if mt % 2 == 0:
                    nc.scalar.activation(
                        ob[:], psums[mt][:], Copy, scale=float(SCALE_X * SCALE_Y)
                    )
                else:
                    nc.vector.tensor_scalar_mul(
                        ob[:], psums[mt][:], float(SCALE_X * SCALE_Y)
                    )
                oeng = nc.gpsimd if (bi >= NB - 1 and mt % 2 == 1) else nc.sync
                oeng.dma_start(out[ts(mt, P), ds(col, w)], ob[:])

    nc.compile()
    return nc


_NC_CACHE = None
LAST_RESULT = None  # BassKernelResults of the most recent run (for profiling)


def _ensure_ntff_hook():
    """concourse's trace path imports antenv.axon_hooks, which is absent
    from this container's antenv stub. Provide it (with the real libaxon
    ctypes hook when available) so tracing works -- or degrades cleanly."""
    import sys
    import types

    try:
        import antenv.axon_hooks  # noqa: F401

        return
    except ImportError:
        pass
    mod = types.ModuleType("antenv.axon_hooks")
    holder = [None]
    mod.set_axon_ntff_profile_hook = lambda h: holder.__setitem__(0, h)
    mod.get_axon_ntff_profile_hook = lambda: holder[0]
    sys.modules["antenv.axon_hooks"] = mod
    try:
        import antenv

        antenv.axon_hooks = mod
    except ImportError:
        pass
    try:
        from trn_agent_boot.trn_boot import _ntff_profile_via_ctypes

        mod.set_axon_ntff_profile_hook(
            _ntff_profile_via_ctypes("/opt/axon/libaxon_pjrt.so")
        )
    except Exception:
        pass  # no hook -> concourse logs a warning and skips tracing


def kernel(x, y):
    global _NC_CACHE, LAST_RESULT
    _ensure_ntff_hook()
    from concourse.bass_utils import run_bass_kernel_spmd

    x = np.asarray(x)
    y = np.asarray(y)
    assert x.shape == (M, K) and y.shape == (K, N), (x.shape, y.shape)
    x8 = x.astype(np.int8) if x.dtype != np.int8 else x
    y8 = y.astype(np.int8) if y.dtype != np.int8 else y

    if _NC_CACHE is None:
        _NC_CACHE = build_nc(M // N_CORES, K, N)
    nc = _NC_CACHE

    m_loc = M // N_CORES
    in_maps = [
        {
            "xt": np.ascontiguousarray(x8[i * m_loc : (i + 1) * m_loc].T),
            "y": y8,
        }
        for i in range(N_CORES)
    ]
    res = run_bass_kernel_spmd(nc, in_maps, core_ids=list(range(N_CORES)))
    LAST_RESULT = res
    return np.concatenate(
        [np.asarray(res.results[i]["out"]) for i in range(N_CORES)], axis=0
    )



# revision 6
# speedup vs baseline: 1.6555x; 1.6555x over previous
"""Quantized int8 matmul on 8 TRN2 NeuronCores.

Math: out = ((x - ZP_X) * SCALE_X) @ ((y - ZP_Y) * SCALE_Y)
Implemented as: out = [(x - ZP_X) @ (y - ZP_Y)] * (SCALE_X * SCALE_Y)
The zero-point-shifted values (range ~[-150, 155]) are rounded to
fp8e4 (e4m3, RNE): per-element relative rounding error ~3% which
averages out over the K=4096 contraction to ~1e-2 output rel err,
within the 2e-2 gate. e4m3 x e4m3 products are exact in the PE's
e10m10 internal format; accumulation is fp32 PSUM. fp8 enables
MatmulPerfMode.DoubleRow: 2 fp8 weights per PE cell -> each matmul
contracts 256 k-values (2 x 128 subtiles) at ~1 col/cycle.

Sharding: x row-sharded (M) across 8 cores, y replicated, no collectives.
Each core's x shard is laid out [K, m_loc] in DRAM (layout chosen at
shard time on host) so the TensorE stationary operand [k-part, m-free]
loads directly -- no on-device transpose.

Engine split per core:
  PE     - warm-up dummies + 512 DoubleRow matmuls (256k x 128m x 512n)
  SP     - HWDGE y loads, first x loads, out store
  GpSimd - SWDGE x loads (keeps the SP sequencer free), warmup memsets
  DVE    - x int8->fp8 +25 converts, y converts (even batches, nb>0),
           odd-mt PSUM evictions
  ACT    - y converts (all of nb=0; odd batches after) + even-mt evictions
"""

import numpy as np

SCALE_X, ZP_X = 0.0215, -25
SCALE_Y, ZP_Y = 0.0176, 18
M, K, N = 4096, 4096, 4096
N_CORES = 8
P = 128
NBLK = 512  # matmul free dim = one PSUM bank of fp32
KB = 2  # k-tiles per y DMA/convert batch
XB = 2  # k-tiles per x DMA/convert batch
N_WARM = 13  # PE warm-up dummy matmuls


def build_nc(m_loc, k, n):
    from contextlib import ExitStack

    import concourse.mybir as mybir
    import concourse.tile as tile
    from concourse import bacc
    from concourse.bass import ds, ts

    fp32 = mybir.dt.float32
    fp8 = mybir.dt.float8e4
    int8 = mybir.dt.int8
    Copy = mybir.ActivationFunctionType.Copy
    DoubleRow = mybir.MatmulPerfMode.DoubleRow

    MT = m_loc // P  # m tiles (PE stationary free dim blocks)
    KT = k // P  # contraction tiles
    NB = n // NBLK  # output column blocks
    kb = min(KB, KT)  # y batch size in k-tiles
    xb = min(XB, KT)  # x batch size in k-tiles
    NKB = KT // kb
    NXB = KT // xb

    nc = bacc.Bacc(None, debug=False)
    xt = nc.declare_dram_parameter("xt", [k, m_loc], int8, isOutput=False)
    y = nc.declare_dram_parameter("y", [k, n], int8, isOutput=False)
    out = nc.declare_dram_parameter("out", [m_loc, n], fp32, isOutput=True)

    # Batched DRAM views: group k into (batch, tile-in-batch, partition)
    xt_r = xt.rearrange("(g b p) m -> g p b m", b=xb, p=P)
    xt_r1 = xt.rearrange("(t p) m -> t p m", p=P)
    y_r = y.rearrange("(q b p) n -> q p b n", b=kb, p=P)

    with ExitStack() as ctx:
        tc = ctx.enter_context(tile.TileContext(nc))
        wm_pool = ctx.enter_context(tc.tile_pool(name="wm", bufs=1))
        xi_pool = ctx.enter_context(tc.tile_pool(name="xi", bufs=4))
        xt_pool = ctx.enter_context(tc.tile_pool(name="xtb", bufs=1, side="right"))
        yi_pool = ctx.enter_context(tc.tile_pool(name="yi", bufs=12))
        yb_pool = ctx.enter_context(tc.tile_pool(name="yb", bufs=12, side="right"))
        ob_pool = ctx.enter_context(tc.tile_pool(name="ob", bufs=4))
        ps_pool = ctx.enter_context(tc.tile_pool(name="ps", bufs=8, space="PSUM"))

        # PE warm-up: a few dummy matmuls on zeroed tiles, issued during
        # the startup DMA window so the HAM clock-gate opens before the
        # real matmul stream begins.
        wm_w = wm_pool.tile([P, P], fp8)
        wm_s = wm_pool.tile([P, NBLK], fp8)
        nc.gpsimd.memset(wm_w[:], 0.0)
        nc.gpsimd.memset(wm_s[:], 0.0)
        ps_warm = ps_pool.tile([P, NBLK], fp32, tag="ps", name="warm")
        for _ in range(N_WARM):
            nc.tensor.matmul(ps_warm[:], wm_w[:], wm_s[:], start=True, stop=True)

        # Persistent fp8 x^T: partition = k within tile, free = (kt, m)
        xT = xt_pool.tile([P, KT, m_loc], fp8)

        def emit_x(g):
            if g >= NXB:
                return
            if g == 0 and xb > 1:
                # first batch as single k-tiles on the fast HWDGE queue so
                # the very first matmul's weights land ASAP
                for t in range(xb):
                    xi = xi_pool.tile([P, m_loc], int8, name=f"xi0_{t}", tag="xi1")
                    nc.sync.dma_start(xi[:], xt_r1[t])
                    nc.vector.tensor_scalar_add(xT[:, t, :], xi[:], float(-ZP_X))
                return
            xi = xi_pool.tile([P, xb, m_loc], int8, name=f"xi_{g}")
            eng = nc.sync if g < 2 else nc.gpsimd
            eng.dma_start(xi[:], xt_r[g])
            nc.vector.tensor_scalar_add(xT[:, ts(g, xb), :], xi[:], float(-ZP_X))

        # Final N-block split into two half-width blocks: the very last
        # eviction + out-DMA drain (which the exit barrier waits on) is
        # half as long.
        blocks = [(i * NBLK, NBLK) for i in range(NB - 1)]
        if NBLK % 2 == 0:
            half = NBLK // 2
            blocks += [((NB - 1) * NBLK, half), ((NB - 1) * NBLK + half, half)]
        else:
            blocks += [((NB - 1) * NBLK, NBLK)]

        for bi, (col, w) in enumerate(blocks):
            psums = [
                ps_pool.tile([P, w], fp32, tag="ps", name=f"acc_{bi}_{i}")
                for i in range(MT)
            ]
            if bi == 0:
                # x batches 0/1 go out first (they gate the first matmuls)
                emit_x(0)
                emit_x(1)
            for q in range(NKB):
                if bi == 0 and q > 0:
                    emit_x(q + 1)
                yi = yi_pool.tile([P, kb, w], int8, name=f"yi_{bi}_{q}", tag="yi")
                nc.sync.dma_start(yi[:], y_r[q, :, :, ds(col, w)])
                yb = yb_pool.tile([P, kb, w], fp8, name=f"yb_{bi}_{q}", tag="yb")
                if bi > 0 and q % 2 == 0:
                    nc.vector.tensor_scalar_add(yb[:], yi[:], float(-ZP_Y))
                else:
                    nc.scalar.activation(yb[:], yi[:], Copy, bias=float(-ZP_Y))
                assert kb == 2, "DoubleRow consumes k-subtile pairs"
                for mt in range(MT):
                    nc.tensor.matmul(
                        psums[mt][:],
                        xT[:, ds(q * kb, kb), ts(mt, P)],
                        yb[:],
                        start=(q == 0),
                        stop=(q == NKB - 1),
                        perf_mode=DoubleRow,
                    )
            for mt in range(MT):
                ob = ob_pool.tile([P, w], fp32, name=f"ob_{bi}_{mt}", tag="ob")
                if mt % 2 == 0:
                    nc.scalar.activation(
                        ob[:], psums[mt][:], Copy, scale=float(SCALE_X * SCALE_Y)
                    )
                else:
                    nc.vector.tensor_scalar_mul(
                        ob[:], psums[mt][:], float(SCALE_X * SCALE_Y)
                    )
                oeng = nc.gpsimd if (bi >= NB - 1 and mt % 2 == 1) else nc.sync
                oeng.dma_start(out[ts(mt, P), ds(col, w)], ob[:])

    nc.compile()
    return nc


_NC_CACHE = None
LAST_RESULT = None  # BassKernelResults of the most recent run (for profiling)


def _ensure_ntff_hook():
    """concourse's trace path imports antenv.axon_hooks, which is absent
    from this container's antenv stub. Provide it (with the real libaxon
    ctypes hook when available) so tracing works -- or degrades cleanly."""
    import sys
    import types

    try:
        import antenv.axon_hooks  # noqa: F401

        return
    except ImportError:
        pass
    mod = types.ModuleType("antenv.axon_hooks")
    holder = [None]
    mod.set_axon_ntff_profile_hook = lambda h: holder.__setitem__(0, h)
    mod.get_axon_ntff_profile_hook = lambda: holder[0]
    sys.modules["antenv.axon_hooks"] = mod
    try:
        import antenv

        antenv.axon_hooks = mod
    except ImportError:
        pass
    try:
        from trn_agent_boot.trn_boot import _ntff_profile_via_ctypes

        mod.set_axon_ntff_profile_hook(
            _ntff_profile_via_ctypes("/opt/axon/libaxon_pjrt.so")
        )
    except Exception:
        pass  # no hook -> concourse logs a warning and skips tracing


def kernel(x, y):
    global _NC_CACHE, LAST_RESULT
    _ensure_ntff_hook()
    from concourse.bass_utils import run_bass_kernel_spmd

    x = np.asarray(x)
    y = np.asarray(y)
    assert x.shape == (M, K) and y.shape == (K, N), (x.shape, y.shape)
    x8 = x.astype(np.int8) if x.dtype != np.int8 else x
    y8 = y.astype(np.int8) if y.dtype != np.int8 else y

    if _NC_CACHE is None:
        _NC_CACHE = build_nc(M // N_CORES, K, N)
    nc = _NC_CACHE

    m_loc = M // N_CORES
    in_maps = [
        {
            "xt": np.ascontiguousarray(x8[i * m_loc : (i + 1) * m_loc].T),
            "y": y8,
        }
        for i in range(N_CORES)
    ]
    res = run_bass_kernel_spmd(nc, in_maps, core_ids=list(range(N_CORES)))
    LAST_RESULT = res
    return np.concatenate(
        [np.asarray(res.results[i]["out"]) for i in range(N_CORES)], axis=0
    )



# revision 9
# speedup vs baseline: 1.6656x; 1.0061x over previous
"""Quantized int8 matmul on 8 TRN2 NeuronCores.

Math: out = ((x - ZP_X) * SCALE_X) @ ((y - ZP_Y) * SCALE_Y)
Implemented as: out = [(x - ZP_X) @ (y - ZP_Y)] * (SCALE_X * SCALE_Y)
The zero-point-shifted values (range ~[-150, 155]) are rounded to
fp8e4 (e4m3, RNE): per-element relative rounding error ~3% which
averages out over the K=4096 contraction to ~1e-2 output rel err,
within the 2e-2 gate. e4m3 x e4m3 products are exact in the PE's
e10m10 internal format; accumulation is fp32 PSUM. fp8 enables
MatmulPerfMode.DoubleRow: 2 fp8 weights per PE cell -> each matmul
contracts 256 k-values (2 x 128 subtiles) at ~1 col/cycle.

Sharding: x row-sharded (M) across 8 cores, y replicated, no collectives.
Each core's x shard is laid out [K, m_loc] in DRAM (layout chosen at
shard time on host) so the TensorE stationary operand [k-part, m-free]
loads directly -- no on-device transpose.

Engine split per core:
  PE     - warm-up dummies + 512 DoubleRow matmuls (256k x 128m x 512n)
  SP     - HWDGE y loads, first x loads, out store
  GpSimd - SWDGE x loads (keeps the SP sequencer free), warmup memsets
  DVE    - x int8->fp8 +25 converts, y converts (even batches, nb>0),
           odd-mt PSUM evictions
  ACT    - y converts (all of nb=0; odd batches after) + even-mt evictions
"""

import numpy as np

SCALE_X, ZP_X = 0.0215, -25
SCALE_Y, ZP_Y = 0.0176, 18
M, K, N = 4096, 4096, 4096
N_CORES = 8
P = 128
NBLK = 512  # matmul free dim = one PSUM bank of fp32
KB = 2  # k-tiles per y DMA/convert batch
XB = 2  # k-tiles per x DMA/convert batch
N_WARM = 8  # PE warm-up dummy matmuls


def build_nc(m_loc, k, n):
    from contextlib import ExitStack

    import concourse.mybir as mybir
    import concourse.tile as tile
    from concourse import bacc
    from concourse.bass import ds, ts

    fp32 = mybir.dt.float32
    fp8 = mybir.dt.float8e4
    int8 = mybir.dt.int8
    Copy = mybir.ActivationFunctionType.Copy
    DoubleRow = mybir.MatmulPerfMode.DoubleRow

    MT = m_loc // P  # m tiles (PE stationary free dim blocks)
    KT = k // P  # contraction tiles
    NB = n // NBLK  # output column blocks
    kb = min(KB, KT)  # y batch size in k-tiles
    xb = min(XB, KT)  # x batch size in k-tiles
    NKB = KT // kb
    NXB = KT // xb

    nc = bacc.Bacc(None, debug=False)
    xt = nc.declare_dram_parameter("xt", [k, m_loc], int8, isOutput=False)
    y = nc.declare_dram_parameter("y", [k, n], int8, isOutput=False)
    out = nc.declare_dram_parameter("out", [m_loc, n], fp32, isOutput=True)

    # Batched DRAM views: group k into (batch, tile-in-batch, partition)
    xt_r = xt.rearrange("(g b p) m -> g p b m", b=xb, p=P)
    xt_r1 = xt.rearrange("(t p) m -> t p m", p=P)
    y_r = y.rearrange("(q b p) n -> q p b n", b=kb, p=P)

    with ExitStack() as ctx:
        tc = ctx.enter_context(tile.TileContext(nc))
        wm_pool = ctx.enter_context(tc.tile_pool(name="wm", bufs=1))
        xi_pool = ctx.enter_context(tc.tile_pool(name="xi", bufs=4))
        xt_pool = ctx.enter_context(tc.tile_pool(name="xtb", bufs=1, side="right"))
        yi_pool = ctx.enter_context(tc.tile_pool(name="yi", bufs=20))
        yb_pool = ctx.enter_context(tc.tile_pool(name="yb", bufs=20, side="right"))
        ob_pool = ctx.enter_context(tc.tile_pool(name="ob", bufs=4))
        ps_pool = ctx.enter_context(tc.tile_pool(name="ps", bufs=8, space="PSUM"))

        # PE warm-up: a few dummy matmuls on zeroed tiles, issued during
        # the startup DMA window so the HAM clock-gate opens before the
        # real matmul stream begins.
        wm_w = wm_pool.tile([P, P], fp8)
        wm_s = wm_pool.tile([P, NBLK], fp8)
        nc.gpsimd.memset(wm_w[:], 0.0)
        nc.gpsimd.memset(wm_s[:], 0.0)
        ps_warm = ps_pool.tile([P, NBLK], fp32, tag="ps", name="warm")
        for _ in range(N_WARM):
            nc.tensor.matmul(ps_warm[:], wm_w[:], wm_s[:], start=True, stop=True)

        # Persistent fp8 x^T: partition = k within tile, free = (kt, m)
        xT = xt_pool.tile([P, KT, m_loc], fp8)

        def emit_x(g):
            if g >= NXB:
                return
            if g == 0 and xb > 1:
                # first batch as single k-tiles on the fast HWDGE queue so
                # the very first matmul's weights land ASAP
                for t in range(xb):
                    xi = xi_pool.tile([P, m_loc], int8, name=f"xi0_{t}", tag="xi1")
                    nc.sync.dma_start(xi[:], xt_r1[t])
                    nc.vector.tensor_scalar_add(xT[:, t, :], xi[:], float(-ZP_X))
                return
            xi = xi_pool.tile([P, xb, m_loc], int8, name=f"xi_{g}")
            eng = nc.sync if g < 2 else nc.gpsimd
            eng.dma_start(xi[:], xt_r[g])
            nc.vector.tensor_scalar_add(xT[:, ts(g, xb), :], xi[:], float(-ZP_X))

        # Final N-block split into two half-width blocks: the very last
        # eviction + out-DMA drain (which the exit barrier waits on) is
        # half as long.
        blocks = [(i * NBLK, NBLK) for i in range(NB - 1)]
        if NBLK % 2 == 0:
            half = NBLK // 2
            blocks += [((NB - 1) * NBLK, half), ((NB - 1) * NBLK + half, half)]
        else:
            blocks += [((NB - 1) * NBLK, NBLK)]

        for bi, (col, w) in enumerate(blocks):
            psums = [
                ps_pool.tile([P, w], fp32, tag="ps", name=f"acc_{bi}_{i}")
                for i in range(MT)
            ]
            if bi == 0:
                # x batches 0/1 go out first (they gate the first matmuls)
                emit_x(0)
                emit_x(1)
            for q in range(NKB):
                if bi == 0 and q > 0:
                    emit_x(q + 1)
                yi = yi_pool.tile([P, kb, w], int8, name=f"yi_{bi}_{q}", tag="yi")
                nc.sync.dma_start(yi[:], y_r[q, :, :, ds(col, w)])
                yb = yb_pool.tile([P, kb, w], fp8, name=f"yb_{bi}_{q}", tag="yb")
                if bi > 0 and q % 2 == 0:
                    nc.vector.tensor_scalar_add(yb[:], yi[:], float(-ZP_Y))
                else:
                    nc.scalar.activation(yb[:], yi[:], Copy, bias=float(-ZP_Y))
                assert kb == 2, "DoubleRow consumes k-subtile pairs"
                for mt in range(MT):
                    nc.tensor.matmul(
                        psums[mt][:],
                        xT[:, ds(q * kb, kb), ts(mt, P)],
                        yb[:],
                        start=(q == 0),
                        stop=(q == NKB - 1),
                        perf_mode=DoubleRow,
                    )
            for mt in range(MT):
                ob = ob_pool.tile([P, w], fp32, name=f"ob_{bi}_{mt}", tag="ob")
                if mt % 2 == 0:
                    nc.scalar.activation(
                        ob[:], psums[mt][:], Copy, scale=float(SCALE_X * SCALE_Y)
                    )
                else:
                    nc.vector.tensor_scalar_mul(
                        ob[:], psums[mt][:], float(SCALE_X * SCALE_Y)
                    )
                # Keep the SP HWDGE ring exclusively for y loads: out stores
                # go on the ACT HWDGE ring (even mt, right after the ACT
                # eviction) or gpsimd SWDGE (odd mt).
                oeng = nc.scalar if mt % 2 == 0 else nc.gpsimd
                oeng.dma_start(out[ts(mt, P), ds(col, w)], ob[:])

    nc.compile()
    return nc


_NC_CACHE = None
LAST_RESULT = None  # BassKernelResults of the most recent run (for profiling)


def _ensure_ntff_hook():
    """concourse's trace path imports antenv.axon_hooks, which is absent
    from this container's antenv stub. Provide it (with the real libaxon
    ctypes hook when available) so tracing works -- or degrades cleanly."""
    import sys
    import types

    try:
        import antenv.axon_hooks  # noqa: F401

        return
    except ImportError:
        pass
    mod = types.ModuleType("antenv.axon_hooks")
    holder = [None]
    mod.set_axon_ntff_profile_hook = lambda h: holder.__setitem__(0, h)
    mod.get_axon_ntff_profile_hook = lambda: holder[0]
    sys.modules["antenv.axon_hooks"] = mod
    try:
        import antenv

        antenv.axon_hooks = mod
    except ImportError:
        pass
    try:
        from trn_agent_boot.trn_boot import _ntff_profile_via_ctypes

        mod.set_axon_ntff_profile_hook(
            _ntff_profile_via_ctypes("/opt/axon/libaxon_pjrt.so")
        )
    except Exception:
        pass  # no hook -> concourse logs a warning and skips tracing


def kernel(x, y):
    global _NC_CACHE, LAST_RESULT
    _ensure_ntff_hook()
    from concourse.bass_utils import run_bass_kernel_spmd

    x = np.asarray(x)
    y = np.asarray(y)
    assert x.shape == (M, K) and y.shape == (K, N), (x.shape, y.shape)
    x8 = x.astype(np.int8) if x.dtype != np.int8 else x
    y8 = y.astype(np.int8) if y.dtype != np.int8 else y

    if _NC_CACHE is None:
        _NC_CACHE = build_nc(M // N_CORES, K, N)
    nc = _NC_CACHE

    m_loc = M // N_CORES
    in_maps = [
        {
            "xt": np.ascontiguousarray(x8[i * m_loc : (i + 1) * m_loc].T),
            "y": y8,
        }
        for i in range(N_CORES)
    ]
    res = run_bass_kernel_spmd(nc, in_maps, core_ids=list(range(N_CORES)))
    LAST_RESULT = res
    return np.concatenate(
        [np.asarray(res.results[i]["out"]) for i in range(N_CORES)], axis=0
    )

